# revision 17
# baseline (speedup 1.0000x reference)
"""Multi-head scaled-dot-product attention (ABSA-style, per-head projections)
on 8 Trainium2 NeuronCores.

Reference computation (per head h, batch b):
    kx = k @ w_kx[h]                    # (512, 96)
    qx = q @ w_qx[h]                    # (512, 96)
    s  = qx @ kx.T / sqrt(96)           # (512, 512)
    a  = softmax(s, axis=-1)
    o  = a @ kx                         # (512, 96)
    out[b, :, h*96:(h+1)*96] = o

Distribution: data-parallel over batch. 32 batches are split 4-per-core over
8 cores; every core holds the full (tiny) weights and computes all 8 heads
for its 4 batches. No collectives needed — the host concatenates the
per-core outputs.

Per-core dataflow (all matmuls in bf16, accumulation + softmax math in f32):
  - SWDGE cast-DMA k,q slices f32->bf16 into a DRAM bounce, then HWDGE
    xbar-transpose-DMA loads kT/qT (embed on partitions) into SBUF.
  - Projections run as 6 accumulating matmuls per (h,b) with the natural
    (embed, hidden) weight layout as the stationary operand, producing
    kx^T/qx^T (hidden, seq) directly — no on-chip weight transposes.
  - Scores are computed transposed, s^T (k, q), so the softmax reduction
    axis lands on PSUM partitions and is folded into the second matmul:
    kx is augmented with a ones column (via a 97-row PE transpose), so the
    attention matmul produces both sum_k exp*kx and sum_k exp (the softmax
    denominator) in one accumulation group.  exp() runs unshifted — scores
    are O(1) by construction so there is no overflow risk — which removes
    the need for any cross-partition max reduction.
  - Normalisation (multiply by the reciprocal of column 96) happens on the
    PSUM->SBUF eviction path into a per-batch staging tile; one contiguous
    DMA per 128 query rows writes all 8 heads at once.
"""

import math
from functools import lru_cache

import numpy as np

import concourse.bass as bass
import concourse.tile as tile
from concourse import mybir
from concourse.bass_utils import run_bass_kernel_spmd
from concourse.masks import make_identity

# ---------------------------------------------------------------------------
# Workaround for walrus "Too many sync wait commands": some instruction
# encodings accept only a single sync-wait, but Tile can attach several
# (e.g. the tail drain, or transpose DMAs gated on both their producer and
# the xbar-mode serialisation).  Hoist every wait beyond the first onto a
# same-engine no-op inserted right before the instruction — program order on
# the engine makes that equivalent.
# ---------------------------------------------------------------------------

import bass_rust as _bass_rust


def _split_excess_waits(nc, max_waits=1):
    n = 0
    for f in nc.m.functions:
        for bb in f.blocks:
            il = bb.instructions
            i = 0
            while i < len(il):
                ins = il[i]
                si = ins.sync_info
                waits = list(si.on_wait or []) if si is not None else []
                if len(waits) > max_waits:
                    si.on_wait = waits[:max_waits]
                    for w in waits[max_waits:]:
                        nop = mybir.InstNoOp(name=f"waitnop-{n}", ins=[],
                                             outs=[])
                        n += 1
                        nop.engine = ins.engine
                        nop.sync_info = _bass_rust.SyncInfo(
                            on_wait=[w], on_update=[])
                        il.insert(i, nop)
                        i += 1
                i += 1

# ---------------------------------------------------------------------------
# Problem constants (full problem; hardcoded per the harness contract)
# ---------------------------------------------------------------------------
EMBED = 768
HID = 96
N_HEAD = 8
BATCH = 32
SEQ = 512
N_CORES = 8
B = BATCH // N_CORES  # batches per core
EC = EMBED // 128  # embed chunks of 128
KC = SEQ // 128  # key chunks of 128
QC = SEQ // 128  # query chunks of 128
SCALE = 1.0 / math.sqrt(HID)

F32 = mybir.dt.float32
BF16 = mybir.dt.bfloat16


def build_bass():
    nc = bass.Bass("TRN2", target_bir_lowering=False, debug=False,
                   num_devices=N_CORES)

    k_in = nc.declare_dram_parameter("k", [B, SEQ, EMBED], F32, isOutput=False)
    q_in = nc.declare_dram_parameter("q", [B, SEQ, EMBED], F32, isOutput=False)
    wk_in = nc.declare_dram_parameter("w_kx", [N_HEAD, EMBED, HID], F32,
                                      isOutput=False)
    wq_in = nc.declare_dram_parameter("w_qx", [N_HEAD, EMBED, HID], F32,
                                      isOutput=False)
    out_d = nc.declare_dram_parameter("out", [B, SEQ, EMBED], F32,
                                      isOutput=True)

    # DRAM bounce for the bf16 copies that feed the xbar transpose loads.
    k_bf = nc.dram_tensor("k_bf", [B, SEQ, EMBED], BF16)
    q_bf = nc.dram_tensor("q_bf", [B, SEQ, EMBED], BF16)

    with nc.allow_low_precision("bf16 compute, f32 accumulate"), \
            tile.TileContext(nc) as tc:
        with tc.tile_pool(name="singles", bufs=1) as singles, \
                tc.tile_pool(name="kqt", bufs=1) as kqt_pool, \
                tc.tile_pool(name="wsb", bufs=1) as w_pool, \
                tc.tile_pool(name="stage", bufs=1) as stage_pool, \
                tc.tile_pool(name="qx", bufs=3) as qx_pool, \
                tc.tile_pool(name="exp", bufs=8) as exp_pool, \
                tc.tile_pool(name="kxo", bufs=3) as kxo_pool, \
                tc.tile_pool(name="recip", bufs=8) as recip_pool, \
                tc.tile_pool(name="ps_proj", bufs=2, space="PSUM") as ps_proj, \
                tc.tile_pool(name="ps_score", bufs=3, space="PSUM") as ps_score, \
                tc.tile_pool(name="ps_trans", bufs=1, space="PSUM") as ps_trans, \
                tc.tile_pool(name="ps_out", bufs=2, space="PSUM") as ps_out:

            # --- one-time setup -------------------------------------------
            identity = singles.tile([128, 128], BF16, tag="identity")
            make_identity(nc, identity[:])

            # kx^T staging tiles with a persistent ones-row (row 96).  Two
            # tiles, alternated per (b,h) iteration for pipelining.
            kx97 = [singles.tile([HID + 1, SEQ], BF16, tag=f"kx97_{i}", name=f"kx97_{i}")
                    for i in range(2)]
            for t in kx97:
                nc.vector.memset(t[HID:HID + 1, :], 1.0)

            # --- input pipeline -------------------------------------------
            # cast f32 -> bf16 (DRAM->DRAM, SWDGE), then xbar transpose-load
            # (embed on partitions).  Batch 0 is queued FIRST so the PE can
            # start as early as possible; the weights (needed by the first
            # matmul too, but small) go right after batch 0's casts, and the
            # remaining batches' casts after that — SWDGE drains its queue in
            # order, and the HWDGE transposes share the same 16 SDMA engines,
            # so front-loading all casts would starve batch 0's transposes.
            # Weights go through HWDGE (own queue, starts immediately and
            # doesn't contend with the SWDGE cast queue) as f32, then one
            # big on-chip DVE cast to bf16.  Laid out (128, 48, 96): block
            # t = 6*h + ec holds head h, embed chunk ec.
            w_sb = [w_pool.tile([128, EC * N_HEAD, HID], BF16,
                                tag=f"w{t}", name=f"wbf{t}")
                    for t in range(2)]
            kT = {}
            qT = {}

            def load_w():
                # w_qx first (the first matmuls are q projections); each
                # tensor in two halves so the DVE cast of half 1 overlaps
                # the load of half 2 and the first head unblocks early.
                # The f32 loads ride the two HWDGE rings before any
                # transposes queue on them.
                HB = EC * N_HEAD // 2
                for t, w_in in ((1, wq_in), (0, wk_in)):
                    eng = nc.sync if t else nc.scalar
                    wf = w_pool.tile([128, EC * N_HEAD, HID], F32,
                                     tag=f"wf{t}", name=f"wf32{t}")
                    src = w_in.rearrange("h e d -> (h e) d").rearrange(
                        "(t p) d -> p t d", p=128)
                    for half in range(2):
                        sl = slice(half * HB, (half + 1) * HB)
                        eng.dma_start(out=wf[:, sl, :], in_=src[:, sl, :])
                        nc.vector.tensor_copy(w_sb[t][:, sl, :], wf[:, sl, :])

            def load_batch(b):
                # q transposes drain on the sync HWDGE ring, k transposes on
                # the scalar ring — the two rings run concurrently, doubling
                # effective transpose throughput.  Casts are SWDGE.
                for t, src, dst_d, eng in ((1, q_in, q_bf, nc.sync),
                                           (0, k_in, k_bf, nc.scalar)):
                    nc.gpsimd.dma_start(out=dst_d[b], in_=src[b])
                    dst = qT if t else kT
                    for ec in range(EC):
                        tt = kqt_pool.tile(
                            [128, SEQ], BF16,
                            tag=f"T{t}_{b}_{ec}", name=f"T{t}_{b}_{ec}")
                        eng.dma_start(
                            out=tt[:],
                            in_=dst_d[b][:, ec * 128:(ec + 1) * 128],
                            transpose=True)
                        dst[b, ec] = tt

            load_w()
            load_batch(0)
            for b in range(1, B):
                load_batch(b)

            # Output staging: (128, EMBED) f32 per (batch parity, q chunk).
            stage = [[stage_pool.tile([128, EMBED], F32, tag=f"st{p}_{qc}", name=f"st{p}_{qc}")
                      for qc in range(QC)] for p in range(2)]

            # --- main loop ------------------------------------------------
            it = 0
            for b in range(B):
                st = stage[b % 2]
                for h in range(N_HEAD):
                    # projections: qx^T and kx^T, (HID, SEQ) f32 in PSUM
                    qx_ps = ps_proj.tile([HID, SEQ], F32, tag="proj", name="proj_ps")
                    for ec in range(EC):
                        nc.tensor.matmul(qx_ps[:], w_sb[1][:, h * EC + ec, :],
                                         qT[b, ec][:],
                                         start=(ec == 0), stop=(ec == EC - 1))
                    qx_sb = qx_pool.tile([HID, SEQ], BF16, tag="qx", name="qx_sb")
                    nc.scalar.copy(qx_sb[:], qx_ps[:])

                    kx_ps = ps_proj.tile([HID, SEQ], F32, tag="proj", name="proj_ps")
                    for ec in range(EC):
                        nc.tensor.matmul(kx_ps[:], w_sb[0][:, h * EC + ec, :],
                                         kT[b, ec][:],
                                         start=(ec == 0), stop=(ec == EC - 1))
                    kx97_t = kx97[it % 2]
                    it += 1
                    nc.vector.tensor_copy(kx97_t[0:HID, :], kx_ps[:])

                    # kx natural layout (+ ones column) via PE transpose
                    tr_ps = ps_trans.tile([128, KC, HID + 2], BF16, tag="tr", name="tr_ps")
                    for kc in range(KC):
                        nc.tensor.transpose(
                            tr_ps[:, kc, 0:HID + 1],
                            kx97_t[:, kc * 128:(kc + 1) * 128],
                            identity[0:HID + 1, 0:HID + 1])
                    kxo = kxo_pool.tile([128, KC, HID + 2], BF16, tag="kxo", name="kxo")
                    nc.vector.tensor_copy(kxo[:, :, 0:HID + 1], tr_ps[:, :, 0:HID + 1])

                    # scores (transposed) + exp
                    exp_sb = []
                    for kc in range(KC):
                        s_ps = ps_score.tile([128, SEQ], F32, tag="score", name="s_ps")
                        nc.tensor.matmul(
                            s_ps[:], kx97_t[0:HID, kc * 128:(kc + 1) * 128],
                            qx_sb[:], start=True, stop=True)
                        e_sb = exp_pool.tile([128, SEQ], BF16, tag="exp", name="e_sb")
                        nc.scalar.activation(
                            e_sb[:], s_ps[:],
                            mybir.ActivationFunctionType.Exp, scale=SCALE)
                        exp_sb.append(e_sb)

                    # attention-weighted values + softmax denominator
                    for qc in range(QC):
                        o_ps = ps_out.tile([128, HID + 1], F32, tag="out", name="o_ps")
                        for kc in range(KC):
                            nc.tensor.matmul(
                                o_ps[:],
                                exp_sb[kc][:, qc * 128:(qc + 1) * 128],
                                kxo[:, kc, 0:HID + 1],
                                start=(kc == 0), stop=(kc == KC - 1))
                        rc = recip_pool.tile([128, 1], F32, tag="recip", name="recip")
                        nc.vector.reciprocal(rc[:], o_ps[:, HID:HID + 1])
                        nc.vector.tensor_scalar_mul(
                            st[qc][:, h * HID:(h + 1) * HID],
                            o_ps[:, 0:HID], rc[:])

                for qc in range(QC):
                    nc.gpsimd.dma_start(
                        out=out_d[b, qc * 128:(qc + 1) * 128, :],
                        in_=st[qc][:])

    _split_excess_waits(nc)
    return nc


@lru_cache(maxsize=1)
def _get_nc():
    return build_bass()


def kernel(k, q, w_kx, w_qx):
    k = np.ascontiguousarray(k, dtype=np.float32)
    q = np.ascontiguousarray(q, dtype=np.float32)
    w_kx = np.ascontiguousarray(w_kx, dtype=np.float32)
    w_qx = np.ascontiguousarray(w_qx, dtype=np.float32)

    nc = _get_nc()
    in_maps = []
    for c in range(N_CORES):
        sl = slice(c * B, (c + 1) * B)
        in_maps.append({
            "k": np.ascontiguousarray(k[sl]),
            "q": np.ascontiguousarray(q[sl]),
            "w_kx": w_kx,
            "w_qx": w_qx,
        })
    res = run_bass_kernel_spmd(nc, in_maps, core_ids=list(range(N_CORES)))
    return np.concatenate([res.results[c]["out"] for c in range(N_CORES)],
                          axis=0)


# revision 18
# speedup vs baseline: 1.0071x; 1.0071x over previous
"""Multi-head scaled-dot-product attention (ABSA-style, per-head projections)
on 8 Trainium2 NeuronCores.

Reference computation (per head h, batch b):
    kx = k @ w_kx[h]                    # (512, 96)
    qx = q @ w_qx[h]                    # (512, 96)
    s  = qx @ kx.T / sqrt(96)           # (512, 512)
    a  = softmax(s, axis=-1)
    o  = a @ kx                         # (512, 96)
    out[b, :, h*96:(h+1)*96] = o

Distribution: data-parallel over batch. 32 batches are split 4-per-core over
8 cores; every core holds the full (tiny) weights and computes all 8 heads
for its 4 batches. No collectives needed — the host concatenates the
per-core outputs.

Per-core dataflow (all matmuls in bf16, accumulation + softmax math in f32):
  - SWDGE cast-DMA k,q slices f32->bf16 into a DRAM bounce, then HWDGE
    xbar-transpose-DMA loads kT/qT (embed on partitions) into SBUF.
  - Projections run as 6 accumulating matmuls per (h,b) with the natural
    (embed, hidden) weight layout as the stationary operand, producing
    kx^T/qx^T (hidden, seq) directly — no on-chip weight transposes.
  - Scores are computed transposed, s^T (k, q), so the softmax reduction
    axis lands on PSUM partitions and is folded into the second matmul:
    kx is augmented with a ones column (via a 97-row PE transpose), so the
    attention matmul produces both sum_k exp*kx and sum_k exp (the softmax
    denominator) in one accumulation group.  exp() runs unshifted — scores
    are O(1) by construction so there is no overflow risk — which removes
    the need for any cross-partition max reduction.
  - Normalisation (multiply by the reciprocal of column 96) happens on the
    PSUM->SBUF eviction path into a per-batch staging tile; one contiguous
    DMA per 128 query rows writes all 8 heads at once.
"""

import math
from functools import lru_cache

import numpy as np

import concourse.bass as bass
import concourse.tile as tile
from concourse import mybir
from concourse.bass_utils import run_bass_kernel_spmd
from concourse.masks import make_identity

# ---------------------------------------------------------------------------
# Workaround for walrus "Too many sync wait commands": some instruction
# encodings accept only a single sync-wait, but Tile can attach several
# (e.g. the tail drain, or transpose DMAs gated on both their producer and
# the xbar-mode serialisation).  Hoist every wait beyond the first onto a
# same-engine no-op inserted right before the instruction — program order on
# the engine makes that equivalent.
# ---------------------------------------------------------------------------

import bass_rust as _bass_rust


def _split_excess_waits(nc, max_waits=1):
    n = 0
    for f in nc.m.functions:
        for bb in f.blocks:
            il = bb.instructions
            i = 0
            while i < len(il):
                ins = il[i]
                si = ins.sync_info
                waits = list(si.on_wait or []) if si is not None else []
                if len(waits) > max_waits:
                    si.on_wait = waits[:max_waits]
                    for w in waits[max_waits:]:
                        nop = mybir.InstNoOp(name=f"waitnop-{n}", ins=[],
                                             outs=[])
                        n += 1
                        nop.engine = ins.engine
                        nop.sync_info = _bass_rust.SyncInfo(
                            on_wait=[w], on_update=[])
                        il.insert(i, nop)
                        i += 1
                i += 1

# ---------------------------------------------------------------------------
# Problem constants (full problem; hardcoded per the harness contract)
# ---------------------------------------------------------------------------
EMBED = 768
HID = 96
N_HEAD = 8
BATCH = 32
SEQ = 512
N_CORES = 8
B = BATCH // N_CORES  # batches per core
EC = EMBED // 128  # embed chunks of 128
KC = SEQ // 128  # key chunks of 128
QC = SEQ // 128  # query chunks of 128
SCALE = 1.0 / math.sqrt(HID)

F32 = mybir.dt.float32
BF16 = mybir.dt.bfloat16


def build_bass():
    nc = bass.Bass("TRN2", target_bir_lowering=False, debug=False,
                   num_devices=N_CORES)

    k_in = nc.declare_dram_parameter("k", [B, SEQ, EMBED], F32, isOutput=False)
    q_in = nc.declare_dram_parameter("q", [B, SEQ, EMBED], F32, isOutput=False)
    wk_in = nc.declare_dram_parameter("w_kx", [N_HEAD, EMBED, HID], F32,
                                      isOutput=False)
    wq_in = nc.declare_dram_parameter("w_qx", [N_HEAD, EMBED, HID], F32,
                                      isOutput=False)
    out_d = nc.declare_dram_parameter("out", [B, SEQ, EMBED], F32,
                                      isOutput=True)

    # DRAM bounce for the bf16 copies that feed the xbar transpose loads.
    k_bf = nc.dram_tensor("k_bf", [B, SEQ, EMBED], BF16)
    q_bf = nc.dram_tensor("q_bf", [B, SEQ, EMBED], BF16)

    with nc.allow_low_precision("bf16 compute, f32 accumulate"), \
            tile.TileContext(nc) as tc:
        with tc.tile_pool(name="singles", bufs=1) as singles, \
                tc.tile_pool(name="kqt", bufs=1) as kqt_pool, \
                tc.tile_pool(name="wsb", bufs=1) as w_pool, \
                tc.tile_pool(name="stage", bufs=1) as stage_pool, \
                tc.tile_pool(name="qx", bufs=3) as qx_pool, \
                tc.tile_pool(name="exp", bufs=8) as exp_pool, \
                tc.tile_pool(name="kxo", bufs=3) as kxo_pool, \
                tc.tile_pool(name="recip", bufs=8) as recip_pool, \
                tc.tile_pool(name="ps_proj", bufs=2, space="PSUM") as ps_proj, \
                tc.tile_pool(name="ps_score", bufs=3, space="PSUM") as ps_score, \
                tc.tile_pool(name="ps_trans", bufs=1, space="PSUM") as ps_trans, \
                tc.tile_pool(name="ps_out", bufs=2, space="PSUM") as ps_out:

            # --- one-time setup -------------------------------------------
            identity = singles.tile([128, 128], BF16, tag="identity")
            make_identity(nc, identity[:])

            # kx^T staging tiles with a persistent ones-row (row 96).  Two
            # tiles, alternated per (b,h) iteration for pipelining.
            kx97 = [singles.tile([HID + 1, SEQ], BF16, tag=f"kx97_{i}", name=f"kx97_{i}")
                    for i in range(2)]
            for t in kx97:
                nc.vector.memset(t[HID:HID + 1, :], 1.0)

            # --- input pipeline -------------------------------------------
            # cast f32 -> bf16 (DRAM->DRAM, SWDGE), then xbar transpose-load
            # (embed on partitions).  Batch 0 is queued FIRST so the PE can
            # start as early as possible; the weights (needed by the first
            # matmul too, but small) go right after batch 0's casts, and the
            # remaining batches' casts after that — SWDGE drains its queue in
            # order, and the HWDGE transposes share the same 16 SDMA engines,
            # so front-loading all casts would starve batch 0's transposes.
            # Weights go through HWDGE (own queue, starts immediately and
            # doesn't contend with the SWDGE cast queue) as f32, then one
            # big on-chip DVE cast to bf16.  Laid out (128, 48, 96): block
            # t = 6*h + ec holds head h, embed chunk ec.
            w_sb = [w_pool.tile([128, EC * N_HEAD, HID], BF16,
                                tag=f"w{t}", name=f"wbf{t}")
                    for t in range(2)]
            kT = {}
            qT = {}

            def load_w():
                # w_qx first (the first matmuls are q projections); each
                # tensor in two halves so the DVE cast of half 1 overlaps
                # the load of half 2 and the first head unblocks early.
                # The f32 loads ride the two HWDGE rings before any
                # transposes queue on them.
                HB = EC * N_HEAD // 2
                for t, w_in in ((1, wq_in), (0, wk_in)):
                    eng = nc.sync if t else nc.scalar
                    wf = w_pool.tile([128, EC * N_HEAD, HID], F32,
                                     tag=f"wf{t}", name=f"wf32{t}")
                    src = w_in.rearrange("h e d -> (h e) d").rearrange(
                        "(t p) d -> p t d", p=128)
                    for half in range(2):
                        sl = slice(half * HB, (half + 1) * HB)
                        eng.dma_start(out=wf[:, sl, :], in_=src[:, sl, :])
                        nc.vector.tensor_copy(w_sb[t][:, sl, :], wf[:, sl, :])

            def load_batch(b):
                # q transposes drain on the sync HWDGE ring, k transposes on
                # the scalar ring — the two rings run concurrently, doubling
                # effective transpose throughput.  Casts are SWDGE.
                for t, src, dst_d, eng in ((1, q_in, q_bf, nc.sync),
                                           (0, k_in, k_bf, nc.scalar)):
                    nc.gpsimd.dma_start(out=dst_d[b], in_=src[b])
                    dst = qT if t else kT
                    for ec in range(EC):
                        tt = kqt_pool.tile(
                            [128, SEQ], BF16,
                            tag=f"T{t}_{b}_{ec}", name=f"T{t}_{b}_{ec}")
                        eng.dma_start(
                            out=tt[:],
                            in_=dst_d[b][:, ec * 128:(ec + 1) * 128],
                            transpose=True)
                        dst[b, ec] = tt

            load_w()
            load_batch(0)
            for b in range(1, B):
                load_batch(b)

            # Output staging: (128, EMBED) f32 per (batch parity, q chunk).
            stage = [[stage_pool.tile([128, EMBED], F32, tag=f"st{p}_{qc}", name=f"st{p}_{qc}")
                      for qc in range(QC)] for p in range(2)]

            # --- main loop ------------------------------------------------
            it = 0
            for b in range(B):
                st = stage[b % 2]
                for h in range(N_HEAD):
                    # projections: qx^T and kx^T, (HID, SEQ) f32 in PSUM
                    qx_ps = ps_proj.tile([HID, SEQ], F32, tag="proj", name="proj_ps")
                    for ec in range(EC):
                        nc.tensor.matmul(qx_ps[:], w_sb[1][:, h * EC + ec, :],
                                         qT[b, ec][:],
                                         start=(ec == 0), stop=(ec == EC - 1))
                    qx_sb = qx_pool.tile([HID, SEQ], BF16, tag="qx", name="qx_sb")
                    nc.scalar.copy(qx_sb[:], qx_ps[:])

                    kx_ps = ps_proj.tile([HID, SEQ], F32, tag="proj", name="proj_ps")
                    for ec in range(EC):
                        nc.tensor.matmul(kx_ps[:], w_sb[0][:, h * EC + ec, :],
                                         kT[b, ec][:],
                                         start=(ec == 0), stop=(ec == EC - 1))
                    kx97_t = kx97[it % 2]
                    it += 1
                    nc.vector.tensor_copy(kx97_t[0:HID, :], kx_ps[:])

                    # kx natural layout (+ ones column) via PE transpose
                    tr_ps = ps_trans.tile([128, KC, HID + 2], BF16, tag="tr", name="tr_ps")
                    for kc in range(KC):
                        nc.tensor.transpose(
                            tr_ps[:, kc, 0:HID + 1],
                            kx97_t[:, kc * 128:(kc + 1) * 128],
                            identity[0:HID + 1, 0:HID + 1])
                    kxo = kxo_pool.tile([128, KC, HID + 2], BF16, tag="kxo", name="kxo")
                    nc.vector.tensor_copy(kxo[:, :, 0:HID + 1], tr_ps[:, :, 0:HID + 1])

                    # scores (transposed) + exp
                    exp_sb = []
                    for kc in range(KC):
                        s_ps = ps_score.tile([128, SEQ], F32, tag="score", name="s_ps")
                        nc.tensor.matmul(
                            s_ps[:], kx97_t[0:HID, kc * 128:(kc + 1) * 128],
                            qx_sb[:], start=True, stop=True)
                        e_sb = exp_pool.tile([128, SEQ], BF16, tag="exp", name="e_sb")
                        nc.scalar.activation(
                            e_sb[:], s_ps[:],
                            mybir.ActivationFunctionType.Exp, scale=SCALE)
                        exp_sb.append(e_sb)

                    # attention-weighted values + softmax denominator
                    for qc in range(QC):
                        o_ps = ps_out.tile([128, HID + 1], F32, tag="out", name="o_ps")
                        for kc in range(KC):
                            nc.tensor.matmul(
                                o_ps[:],
                                exp_sb[kc][:, qc * 128:(qc + 1) * 128],
                                kxo[:, kc, 0:HID + 1],
                                start=(kc == 0), stop=(kc == KC - 1))
                        rc = recip_pool.tile([128, 1], F32, tag="recip", name="recip")
                        nc.vector.reciprocal(rc[:], o_ps[:, HID:HID + 1])
                        nc.vector.tensor_scalar_mul(
                            st[qc][:, h * HID:(h + 1) * HID],
                            o_ps[:, 0:HID], rc[:])

                for qc in range(QC):
                    nc.sync.dma_start(
                        out=out_d[b, qc * 128:(qc + 1) * 128, :],
                        in_=st[qc][:])

    _split_excess_waits(nc)
    return nc


@lru_cache(maxsize=1)
def _get_nc():
    return build_bass()


def kernel(k, q, w_kx, w_qx):
    k = np.ascontiguousarray(k, dtype=np.float32)
    q = np.ascontiguousarray(q, dtype=np.float32)
    w_kx = np.ascontiguousarray(w_kx, dtype=np.float32)
    w_qx = np.ascontiguousarray(w_qx, dtype=np.float32)

    nc = _get_nc()
    in_maps = []
    for c in range(N_CORES):
        sl = slice(c * B, (c + 1) * B)
        in_maps.append({
            "k": np.ascontiguousarray(k[sl]),
            "q": np.ascontiguousarray(q[sl]),
            "w_kx": w_kx,
            "w_qx": w_qx,
        })
    res = run_bass_kernel_spmd(nc, in_maps, core_ids=list(range(N_CORES)))
    return np.concatenate([res.results[c]["out"] for c in range(N_CORES)],
                          axis=0)


# revision 19
# speedup vs baseline: 1.0453x; 1.0379x over previous
"""Multi-head scaled-dot-product attention (ABSA-style, per-head projections)
on 8 Trainium2 NeuronCores.

Reference computation (per head h, batch b):
    kx = k @ w_kx[h]                    # (512, 96)
    qx = q @ w_qx[h]                    # (512, 96)
    s  = qx @ kx.T / sqrt(96)           # (512, 512)
    a  = softmax(s, axis=-1)
    o  = a @ kx                         # (512, 96)
    out[b, :, h*96:(h+1)*96] = o

Distribution: data-parallel over batch. 32 batches are split 4-per-core over
8 cores; every core holds the full (tiny) weights and computes all 8 heads
for its 4 batches. No collectives needed — the host concatenates the
per-core outputs.

Per-core dataflow (all matmuls in bf16, accumulation + softmax math in f32):
  - SWDGE cast-DMA k,q slices f32->bf16 into a DRAM bounce, then HWDGE
    xbar-transpose-DMA loads kT/qT (embed on partitions) into SBUF.
  - Projections run as 6 accumulating matmuls per (h,b) with the natural
    (embed, hidden) weight layout as the stationary operand, producing
    kx^T/qx^T (hidden, seq) directly — no on-chip weight transposes.
  - Scores are computed transposed, s^T (k, q), so the softmax reduction
    axis lands on PSUM partitions and is folded into the second matmul:
    kx is augmented with a ones column (via a 97-row PE transpose), so the
    attention matmul produces both sum_k exp*kx and sum_k exp (the softmax
    denominator) in one accumulation group.  exp() runs unshifted — scores
    are O(1) by construction so there is no overflow risk — which removes
    the need for any cross-partition max reduction.
  - Normalisation (multiply by the reciprocal of column 96) happens on the
    PSUM->SBUF eviction path into a per-batch staging tile; one contiguous
    DMA per 128 query rows writes all 8 heads at once.
"""

import math
from functools import lru_cache

import numpy as np

import concourse.bass as bass
import concourse.tile as tile
from concourse import mybir
from concourse.bass_utils import run_bass_kernel_spmd
from concourse.masks import make_identity

# ---------------------------------------------------------------------------
# Workaround for walrus "Too many sync wait commands": some instruction
# encodings accept only a single sync-wait, but Tile can attach several
# (e.g. the tail drain, or transpose DMAs gated on both their producer and
# the xbar-mode serialisation).  Hoist every wait beyond the first onto a
# same-engine no-op inserted right before the instruction — program order on
# the engine makes that equivalent.
# ---------------------------------------------------------------------------

import bass_rust as _bass_rust


def _split_excess_waits(nc, max_waits=1):
    n = 0
    for f in nc.m.functions:
        for bb in f.blocks:
            il = bb.instructions
            i = 0
            while i < len(il):
                ins = il[i]
                si = ins.sync_info
                waits = list(si.on_wait or []) if si is not None else []
                if len(waits) > max_waits:
                    si.on_wait = waits[:max_waits]
                    for w in waits[max_waits:]:
                        nop = mybir.InstNoOp(name=f"waitnop-{n}", ins=[],
                                             outs=[])
                        n += 1
                        nop.engine = ins.engine
                        nop.sync_info = _bass_rust.SyncInfo(
                            on_wait=[w], on_update=[])
                        il.insert(i, nop)
                        i += 1
                i += 1

# ---------------------------------------------------------------------------
# Problem constants (full problem; hardcoded per the harness contract)
# ---------------------------------------------------------------------------
EMBED = 768
HID = 96
N_HEAD = 8
BATCH = 32
SEQ = 512
N_CORES = 8
B = BATCH // N_CORES  # batches per core
EC = EMBED // 128  # embed chunks of 128
KC = SEQ // 128  # key chunks of 128
QC = SEQ // 128  # query chunks of 128
SCALE = 1.0 / math.sqrt(HID)

F32 = mybir.dt.float32
BF16 = mybir.dt.bfloat16


def build_bass():
    nc = bass.Bass("TRN2", target_bir_lowering=False, debug=False,
                   num_devices=N_CORES)

    k_in = nc.declare_dram_parameter("k", [B, SEQ, EMBED], F32, isOutput=False)
    q_in = nc.declare_dram_parameter("q", [B, SEQ, EMBED], F32, isOutput=False)
    wk_in = nc.declare_dram_parameter("w_kx", [N_HEAD, EMBED, HID], F32,
                                      isOutput=False)
    wq_in = nc.declare_dram_parameter("w_qx", [N_HEAD, EMBED, HID], F32,
                                      isOutput=False)
    out_d = nc.declare_dram_parameter("out", [B, SEQ, EMBED], F32,
                                      isOutput=True)

    # DRAM bounce for the bf16 copies that feed the xbar transpose loads.
    # One tensor PER BATCH: a single [B, ...] tensor makes Tile's coarse
    # DRAM dependency tracking serialise batch b+1's cast behind batch b's
    # transpose reads (false WAR), which strangles the input pipeline.
    k_bf = [nc.dram_tensor(f"k_bf{b}", [SEQ, EMBED], BF16) for b in range(B)]
    q_bf = [nc.dram_tensor(f"q_bf{b}", [SEQ, EMBED], BF16) for b in range(B)]

    with nc.allow_low_precision("bf16 compute, f32 accumulate"), \
            tile.TileContext(nc) as tc:
        with tc.tile_pool(name="singles", bufs=1) as singles, \
                tc.tile_pool(name="kqt", bufs=1) as kqt_pool, \
                tc.tile_pool(name="wsb", bufs=1) as w_pool, \
                tc.tile_pool(name="stage", bufs=1) as stage_pool, \
                tc.tile_pool(name="qx", bufs=3) as qx_pool, \
                tc.tile_pool(name="exp", bufs=8) as exp_pool, \
                tc.tile_pool(name="kxo", bufs=3) as kxo_pool, \
                tc.tile_pool(name="recip", bufs=8) as recip_pool, \
                tc.tile_pool(name="ps_proj", bufs=2, space="PSUM") as ps_proj, \
                tc.tile_pool(name="ps_score", bufs=3, space="PSUM") as ps_score, \
                tc.tile_pool(name="ps_trans", bufs=1, space="PSUM") as ps_trans, \
                tc.tile_pool(name="ps_out", bufs=2, space="PSUM") as ps_out:

            # --- one-time setup -------------------------------------------
            identity = singles.tile([128, 128], BF16, tag="identity")
            make_identity(nc, identity[:])

            # kx^T staging tiles with a persistent ones-row (row 96).  Two
            # tiles, alternated per (b,h) iteration for pipelining.
            kx97 = [singles.tile([HID + 1, SEQ], BF16, tag=f"kx97_{i}", name=f"kx97_{i}")
                    for i in range(2)]
            for t in kx97:
                nc.vector.memset(t[HID:HID + 1, :], 1.0)

            # --- input pipeline -------------------------------------------
            # cast f32 -> bf16 (DRAM->DRAM, SWDGE), then xbar transpose-load
            # (embed on partitions).  Batch 0 is queued FIRST so the PE can
            # start as early as possible; the weights (needed by the first
            # matmul too, but small) go right after batch 0's casts, and the
            # remaining batches' casts after that — SWDGE drains its queue in
            # order, and the HWDGE transposes share the same 16 SDMA engines,
            # so front-loading all casts would starve batch 0's transposes.
            # Weights go through HWDGE (own queue, starts immediately and
            # doesn't contend with the SWDGE cast queue) as f32, then one
            # big on-chip DVE cast to bf16.  Laid out (128, 48, 96): block
            # t = 6*h + ec holds head h, embed chunk ec.
            w_sb = [w_pool.tile([128, EC * N_HEAD, HID], BF16,
                                tag=f"w{t}", name=f"wbf{t}")
                    for t in range(2)]
            kT = {}
            qT = {}

            def load_w():
                # w_qx first (the first matmuls are q projections); each
                # tensor in two halves so the DVE cast of half 1 overlaps
                # the load of half 2 and the first head unblocks early.
                # The f32 loads ride the two HWDGE rings before any
                # transposes queue on them.
                HB = EC * N_HEAD // 2
                for t, w_in in ((1, wq_in), (0, wk_in)):
                    eng = nc.sync if t else nc.scalar
                    wf = w_pool.tile([128, EC * N_HEAD, HID], F32,
                                     tag=f"wf{t}", name=f"wf32{t}")
                    src = w_in.rearrange("h e d -> (h e) d").rearrange(
                        "(t p) d -> p t d", p=128)
                    for half in range(2):
                        sl = slice(half * HB, (half + 1) * HB)
                        eng.dma_start(out=wf[:, sl, :], in_=src[:, sl, :])
                        nc.vector.tensor_copy(w_sb[t][:, sl, :], wf[:, sl, :])

            def load_batch(b):
                # q transposes drain on the sync HWDGE ring, k transposes on
                # the scalar ring — the two rings run concurrently, doubling
                # effective transpose throughput.  Casts are SWDGE.
                for t, src, dst_d, eng in ((1, q_in, q_bf, nc.sync),
                                           (0, k_in, k_bf, nc.scalar)):
                    nc.gpsimd.dma_start(out=dst_d[b][:], in_=src[b])
                    dst = qT if t else kT
                    for ec in range(EC):
                        tt = kqt_pool.tile(
                            [128, SEQ], BF16,
                            tag=f"T{t}_{b}_{ec}", name=f"T{t}_{b}_{ec}")
                        eng.dma_start(
                            out=tt[:],
                            in_=dst_d[b][:, ec * 128:(ec + 1) * 128],
                            transpose=True)
                        dst[b, ec] = tt

            load_w()
            load_batch(0)
            for b in range(1, B):
                load_batch(b)

            # Output staging: (128, EMBED) f32 per (batch parity, q chunk).
            stage = [[stage_pool.tile([128, EMBED], F32, tag=f"st{p}_{qc}", name=f"st{p}_{qc}")
                      for qc in range(QC)] for p in range(2)]

            # --- main loop ------------------------------------------------
            it = 0
            for b in range(B):
                st = stage[b % 2]
                for h in range(N_HEAD):
                    # projections: qx^T and kx^T, (HID, SEQ) f32 in PSUM
                    qx_ps = ps_proj.tile([HID, SEQ], F32, tag="proj", name="proj_ps")
                    for ec in range(EC):
                        nc.tensor.matmul(qx_ps[:], w_sb[1][:, h * EC + ec, :],
                                         qT[b, ec][:],
                                         start=(ec == 0), stop=(ec == EC - 1))
                    qx_sb = qx_pool.tile([HID, SEQ], BF16, tag="qx", name="qx_sb")
                    nc.scalar.copy(qx_sb[:], qx_ps[:])

                    kx_ps = ps_proj.tile([HID, SEQ], F32, tag="proj", name="proj_ps")
                    for ec in range(EC):
                        nc.tensor.matmul(kx_ps[:], w_sb[0][:, h * EC + ec, :],
                                         kT[b, ec][:],
                                         start=(ec == 0), stop=(ec == EC - 1))
                    kx97_t = kx97[it % 2]
                    it += 1
                    nc.vector.tensor_copy(kx97_t[0:HID, :], kx_ps[:])

                    # kx natural layout (+ ones column) via PE transpose
                    tr_ps = ps_trans.tile([128, KC, HID + 2], BF16, tag="tr", name="tr_ps")
                    for kc in range(KC):
                        nc.tensor.transpose(
                            tr_ps[:, kc, 0:HID + 1],
                            kx97_t[:, kc * 128:(kc + 1) * 128],
                            identity[0:HID + 1, 0:HID + 1])
                    kxo = kxo_pool.tile([128, KC, HID + 2], BF16, tag="kxo", name="kxo")
                    nc.vector.tensor_copy(kxo[:, :, 0:HID + 1], tr_ps[:, :, 0:HID + 1])

                    # scores (transposed) + exp
                    exp_sb = []
                    for kc in range(KC):
                        s_ps = ps_score.tile([128, SEQ], F32, tag="score", name="s_ps")
                        nc.tensor.matmul(
                            s_ps[:], kx97_t[0:HID, kc * 128:(kc + 1) * 128],
                            qx_sb[:], start=True, stop=True)
                        e_sb = exp_pool.tile([128, SEQ], BF16, tag="exp", name="e_sb")
                        nc.scalar.activation(
                            e_sb[:], s_ps[:],
                            mybir.ActivationFunctionType.Exp, scale=SCALE)
                        exp_sb.append(e_sb)

                    # attention-weighted values + softmax denominator
                    for qc in range(QC):
                        o_ps = ps_out.tile([128, HID + 1], F32, tag="out", name="o_ps")
                        for kc in range(KC):
                            nc.tensor.matmul(
                                o_ps[:],
                                exp_sb[kc][:, qc * 128:(qc + 1) * 128],
                                kxo[:, kc, 0:HID + 1],
                                start=(kc == 0), stop=(kc == KC - 1))
                        rc = recip_pool.tile([128, 1], F32, tag="recip", name="recip")
                        nc.vector.reciprocal(rc[:], o_ps[:, HID:HID + 1])
                        nc.vector.tensor_scalar_mul(
                            st[qc][:, h * HID:(h + 1) * HID],
                            o_ps[:, 0:HID], rc[:])

                for qc in range(QC):
                    nc.sync.dma_start(
                        out=out_d[b, qc * 128:(qc + 1) * 128, :],
                        in_=st[qc][:])

    _split_excess_waits(nc)
    return nc


@lru_cache(maxsize=1)
def _get_nc():
    return build_bass()


def kernel(k, q, w_kx, w_qx):
    k = np.ascontiguousarray(k, dtype=np.float32)
    q = np.ascontiguousarray(q, dtype=np.float32)
    w_kx = np.ascontiguousarray(w_kx, dtype=np.float32)
    w_qx = np.ascontiguousarray(w_qx, dtype=np.float32)

    nc = _get_nc()
    in_maps = []
    for c in range(N_CORES):
        sl = slice(c * B, (c + 1) * B)
        in_maps.append({
            "k": np.ascontiguousarray(k[sl]),
            "q": np.ascontiguousarray(q[sl]),
            "w_kx": w_kx,
            "w_qx": w_qx,
        })
    res = run_bass_kernel_spmd(nc, in_maps, core_ids=list(range(N_CORES)))
    return np.concatenate([res.results[c]["out"] for c in range(N_CORES)],
                          axis=0)
